# revision 6
# baseline (speedup 1.0000x reference)
"""Trainium2 Bass kernel for DSF marginal (MLP conditioner + deep sigmoid flow).

Contract: kernel(**inputs) takes the FULL unsharded inputs of reference.setup_inputs()
and returns (out, logdet), both [32, 4096] float32, matching reference.reference().

Sharding: data-parallel over rows (B*N = 131072) -> 8 cores x 16384 rows.
Device program (per core):
  - MLP: h1 = relu(ctx@W1+b1), h2 = relu(h1@W2+b2), params = h2@W3+b3, in bf16
    matmuls (fp32 PSUM accumulate), feature-major chaining; context arrives
    pre-transposed/cast from the host so no on-chip transposes are needed.
  - Flow: 6 sigmoid-flow layers computed with only Exp/Ln/Relu/Copy ACT
    functions (single ACT table set), elementwise work spread across
    DVE / ACT / GPSIMD.
"""

import functools

import numpy as np
import ml_dtypes

import concourse.bass as bass
import concourse.tile as tile
from concourse import mybir
from concourse.bass_utils import run_bass_kernel_spmd

F32 = mybir.dt.float32
BF16 = mybir.dt.bfloat16
AF = mybir.ActivationFunctionType
OP = mybir.AluOpType
AX = mybir.AxisListType

H = 48
N_LAYERS = 6
EPS = 1e-6
T_FLOW = 3 * H  # 144 params per layer
T_ALL = T_FLOW * N_LAYERS  # 864
C_IN = 256
D_HID = 512
B, N = 32, 4096
N_CORES = 8
ROWS_CORE = B * N // N_CORES  # 16384
SUB = 512                     # rows per matmul sub-block (PSUM N)
BLK = 2048                    # rows per flow block
T = BLK // 128                # 16 row-tiles per block
N_BLK = ROWS_CORE // BLK      # 8
N_SUB = BLK // SUB            # 4 sub-blocks per block

# This walrus build rejects instructions carrying more than one sem-wait
# ("Too many sync wait commands" in setupSyncWait). Split excess waits onto
# NOPs inserted just before the instruction on the same engine.
_MAX_WAITS = 1


def _split_excess_waits(nc):
    import bass_rust
    n_split = 0
    for f in nc.m.functions:
        for bb in f.blocks:
            out = []
            for inst in bb.instructions:
                si = getattr(inst, "sync_info", None)
                ow = list(si.on_wait) if (si is not None and si.on_wait) else []
                if len(ow) > _MAX_WAITS:
                    extra, keep = ow[:-_MAX_WAITS], ow[-_MAX_WAITS:]
                    for j in range(0, len(extra), _MAX_WAITS):
                        n_split += 1
                        nop = bass_rust.InstNoOp(
                            name=f"I-wsplit-{n_split}", engine=inst.engine,
                            ins=[], outs=[])
                        nop.sync_info = mybir.SyncInfo(
                            on_wait=list(extra[j:j + _MAX_WAITS]), on_update=[])
                        out.append(nop)
                    si.on_wait[:] = keep
                out.append(inst)
            bb.instructions = out
    return n_split


def build_nc(n_blk=N_BLK, num_devices=N_CORES, repeat=1):
    nc = bass.Bass(trn_type="TRN2", num_devices=num_devices)

    n_sub_total = n_blk * N_SUB
    ctxT_d = nc.dram_tensor("ctxT", [n_sub_total, 128, 2, SUB], BF16, kind="ExternalInput")
    xv_d = nc.dram_tensor("xv", [n_blk, 128, T], F32, kind="ExternalInput")
    w1_d = nc.dram_tensor("w1", [128, 2, D_HID], BF16, kind="ExternalInput")
    w2_d = nc.dram_tensor("w2", [128, 4, D_HID], BF16, kind="ExternalInput")
    w3_d = nc.dram_tensor("w3", [128, 4, T_ALL], BF16, kind="ExternalInput")
    b1_d = nc.dram_tensor("b1c", [128, 4], F32, kind="ExternalInput")
    b2_d = nc.dram_tensor("b2c", [128, 4], F32, kind="ExternalInput")
    b3_d = nc.dram_tensor("b3c", [1, T_ALL], BF16, kind="ExternalInput")
    o_d = nc.dram_tensor("o", [n_blk, 128, T], F32, kind="ExternalOutput")
    ld_d = nc.dram_tensor("ld", [n_blk, 128, T], F32, kind="ExternalOutput")

    with tile.TileContext(nc) as tc:
        with (
            tc.tile_pool(name="weights", bufs=1) as wpool,
            tc.tile_pool(name="ctx", bufs=3) as ctxp,
            tc.tile_pool(name="hbuf", bufs=2) as hpool,
            tc.tile_pool(name="params", bufs=2) as ppool,
            tc.tile_pool(name="psh", bufs=3, space="PSUM") as psh,
            tc.tile_pool(name="psp", bufs=2, space="PSUM") as psp,
            tc.tile_pool(name="flow32", bufs=1) as f32p,
            tc.tile_pool(name="flow16", bufs=2) as f16p,
            tc.tile_pool(name="rows", bufs=2) as rowp,
        ):
            w1 = wpool.tile([128, 2, D_HID], BF16, tag="w1")
            nc.sync.dma_start(out=w1[:, :, :], in_=w1_d[:, :, :])
            w2 = wpool.tile([128, 4, D_HID], BF16, tag="w2")
            nc.sync.dma_start(out=w2[:, :, :], in_=w2_d[:, :, :])
            w3 = wpool.tile([128, 4, T_ALL], BF16, tag="w3")
            nc.sync.dma_start(out=w3[:, :, :], in_=w3_d[:, :, :])
            b1s = wpool.tile([128, 4], F32, tag="b1s")
            nc.sync.dma_start(out=b1s[:, :], in_=b1_d[:, :])
            b2s = wpool.tile([128, 4], F32, tag="b2s")
            nc.sync.dma_start(out=b2s[:, :], in_=b2_d[:, :])
            b3s = wpool.tile([1, T_ALL], BF16, tag="b3s")
            nc.sync.dma_start(out=b3s[:, :], in_=b3_d[:, :])
            ones = wpool.tile([1, 128], BF16, tag="ones")
            nc.vector.memset(ones[:, :], 1.0)

            for b in [b for _ in range(repeat) for b in range(n_blk)]:
                P = ppool.tile([128, T, T_ALL], F32, tag="P")

                for s in range(N_SUB):
                    ct = ctxp.tile([128, 2, SUB], BF16, tag="ct")
                    nc.sync.dma_start(out=ct[:, :, :], in_=ctxT_d[b * N_SUB + s, :, :, :])

                    h1 = hpool.tile([128, 4, SUB], BF16, tag="h1")
                    for m in range(4):
                        ps = psh.tile([128, SUB], F32, tag="psh")
                        for k in range(2):
                            nc.tensor.matmul(
                                ps[:, :],
                                lhsT=w1[:, k, m * 128:(m + 1) * 128],
                                rhs=ct[:, k, :],
                                start=(k == 0), stop=(k == 1),
                            )
                        if m < 2:
                            nc.scalar.activation(h1[:, m, :], ps[:, :], AF.Relu,
                                                 bias=b1s[:, m:m + 1], scale=1.0)
                        else:
                            nc.vector.tensor_scalar(
                                out=h1[:, m, :], in0=ps[:, :],
                                scalar1=b1s[:, m:m + 1], scalar2=0.0,
                                op0=OP.add, op1=OP.max,
                            )

                    h2 = hpool.tile([128, 4, SUB], BF16, tag="h2")
                    for m in range(4):
                        ps = psh.tile([128, SUB], F32, tag="psh")
                        for k in range(4):
                            nc.tensor.matmul(
                                ps[:, :],
                                lhsT=w2[:, k, m * 128:(m + 1) * 128],
                                rhs=h1[:, k, :],
                                start=(k == 0), stop=(k == 3),
                            )
                        if m < 2:
                            nc.scalar.activation(h2[:, m, :], ps[:, :], AF.Relu,
                                                 bias=b2s[:, m:m + 1], scale=1.0)
                        else:
                            nc.vector.tensor_scalar(
                                out=h2[:, m, :], in0=ps[:, :],
                                scalar1=b2s[:, m:m + 1], scalar2=0.0,
                                op0=OP.add, op1=OP.max,
                            )

                    for r in range(4):
                        pp = psp.tile([128, T_ALL], F32, tag="psp")
                        for lo, hi in ((0, 512), (512, T_ALL)):
                            for k in range(4):
                                nc.tensor.matmul(
                                    pp[:, lo:hi],
                                    lhsT=h2[:, k, r * 128:(r + 1) * 128],
                                    rhs=w3[:, k, lo:hi],
                                    start=(k == 0), stop=False,
                                )
                            nc.tensor.matmul(
                                pp[:, lo:hi],
                                lhsT=ones[:, :],
                                rhs=b3s[:, lo:hi],
                                start=False, stop=True,
                            )
                        idx = s * 4 + r
                        if r < 2:
                            nc.vector.tensor_copy(out=P[:, idx, :], in_=pp[:, :])
                        else:
                            nc.scalar.copy(P[:, idx, :], pp[:, :])

                # ---- sigmoid flow over this block ----
                xt = rowp.tile([128, T], F32, tag="x")
                nc.sync.dma_start(out=xt[:, :], in_=xv_d[b, :, :])
                ld = rowp.tile([128, T], F32, tag="ldet")

                xpre = None
                for i in range(N_LAYERS):
                    o0 = i * T_FLOW
                    Pa = P[:, :, o0:o0 + H]
                    Pb = P[:, :, o0 + H:o0 + 2 * H]
                    Pw = P[:, :, o0 + 2 * H:o0 + 3 * H]

                    ea = f32p.tile([128, T, H], F32, tag="ea")
                    nc.scalar.activation(ea[:, :, :], Pa, AF.Exp)
                    a = f16p.tile([128, T, H], BF16, tag="a")
                    nc.scalar.activation(a[:, :, :], ea[:, :, :], AF.Ln, bias=1.0)

                    t_ = f32p.tile([128, T, H], F32, tag="t")
                    nc.vector.tensor_tensor(
                        t_[:, :, :], a[:, :, :],
                        xt[:, :, None].broadcast_to([128, T, H]), OP.mult)
                    pre = f32p.tile([128, T, H], F32, tag="pre")
                    nc.vector.tensor_tensor(pre[:, :, :], t_[:, :, :], Pb, OP.add)

                    en = f16p.tile([128, T, H], BF16, tag="en")
                    nc.scalar.activation(en[:, :, :], pre[:, :, :], AF.Exp, scale=-1.0)
                    s_ = f32p.tile([128, T, H], F32, tag="s")
                    nc.scalar.activation(s_[:, :, :], en[:, :, :], AF.Ln, bias=1.0)
                    sig = f16p.tile([128, T, H], BF16, tag="sig")
                    nc.scalar.activation(sig[:, :, :], s_[:, :, :], AF.Exp, scale=-1.0)
                    e = f16p.tile([128, T, H], BF16, tag="e")
                    nc.scalar.activation(e[:, :, :], Pw, AF.Exp)

                    sign = f16p.tile([128, T, H], BF16, tag="sign")
                    nc.gpsimd.tensor_tensor(sign[:, :, :], en[:, :, :], sig[:, :, :], OP.mult)
                    prod = f16p.tile([128, T, H], BF16, tag="prod")
                    nc.vector.tensor_tensor(prod[:, :, :], e[:, :, :], sig[:, :, :], OP.mult)
                    q1 = f16p.tile([128, T, H], BF16, tag="q1")
                    nc.gpsimd.tensor_tensor(q1[:, :, :], prod[:, :, :], a[:, :, :], OP.mult)
                    ej = f16p.tile([128, T, H], BF16, tag="ej")
                    nc.gpsimd.tensor_tensor(ej[:, :, :], q1[:, :, :], sign[:, :, :], OP.mult)

                    sum_e = rowp.tile([128, T], F32, tag="sum_e")
                    nc.vector.reduce_sum(out=sum_e[:, :], in_=e[:, :, :], axis=AX.X)
                    dot = rowp.tile([128, T], F32, tag="dot")
                    nc.vector.reduce_sum(out=dot[:, :], in_=prod[:, :, :], axis=AX.X)
                    sj = rowp.tile([128, T], F32, tag="sj")
                    nc.vector.reduce_sum(out=sj[:, :], in_=ej[:, :, :], axis=AX.X)

                    rec = rowp.tile([128, T], F32, tag="rec")
                    nc.vector.reciprocal(rec[:, :], sum_e[:, :])
                    xpre = rowp.tile([128, T], F32, tag="xpre")
                    nc.vector.tensor_mul(xpre[:, :], dot[:, :], rec[:, :])
                    u = rowp.tile([128, T], F32, tag="u")
                    nc.vector.tensor_mul(u[:, :], sj[:, :], rec[:, :])
                    lse = rowp.tile([128, T], F32, tag="lse")
                    nc.scalar.activation(lse[:, :], u[:, :], AF.Ln)
                    if i == 0:
                        nc.vector.tensor_copy(out=ld[:, :], in_=lse[:, :])
                    else:
                        nc.vector.tensor_add(ld[:, :], ld[:, :], lse[:, :])

                    if i < N_LAYERS - 1:
                        xc = rowp.tile([128, T], F32, tag="xc")
                        nc.vector.tensor_scalar(
                            out=xc[:, :], in0=xpre[:, :],
                            scalar1=1.0 - EPS, scalar2=EPS * 0.5,
                            op0=OP.mult, op1=OP.add,
                        )
                        lxc = rowp.tile([128, T], F32, tag="lxc")
                        nc.scalar.activation(lxc[:, :], xc[:, :], AF.Ln)
                        l1m = rowp.tile([128, T], F32, tag="l1m")
                        nc.scalar.activation(l1m[:, :], xc[:, :], AF.Ln, scale=-1.0, bias=1.0)
                        nc.vector.tensor_sub(xt[:, :], lxc[:, :], l1m[:, :])
                        wsum = rowp.tile([128, T], F32, tag="wsum")
                        nc.vector.tensor_add(wsum[:, :], lxc[:, :], l1m[:, :])
                        nc.vector.tensor_sub(ld[:, :], ld[:, :], wsum[:, :])

                # +5*log(1-eps) from the 5 logit layers
                nc.vector.tensor_scalar_add(ld[:, :], ld[:, :],
                                            float(5.0 * np.log1p(-EPS)))
                nc.sync.dma_start(out=o_d[b, :, :], in_=xpre[:, :])
                nc.sync.dma_start(out=ld_d[b, :, :], in_=ld[:, :])

    _split_excess_waits(nc)
    return nc


@functools.lru_cache(maxsize=2)
def _get_nc():
    return build_nc()


def _prep_core(ctx_core, x_core, n_blk=N_BLK):
    """Host-side layout: ctx_core [rows, 256] f32, x_core [rows] f32."""
    n_sub_total = n_blk * N_SUB
    ctxT = ctx_core.reshape(n_sub_total, SUB, 2, 128).transpose(0, 3, 2, 1)
    ctxT = np.ascontiguousarray(ctxT).astype(ml_dtypes.bfloat16)
    xv = np.ascontiguousarray(x_core.reshape(n_blk, T, 128).transpose(0, 2, 1))
    return ctxT, xv


def _prep_weights(W1, b1, W2, b2, W3, b3):
    bf = ml_dtypes.bfloat16
    w1 = np.ascontiguousarray(W1.reshape(2, 128, D_HID).transpose(1, 0, 2)).astype(bf)
    w2 = np.ascontiguousarray(W2.reshape(4, 128, D_HID).transpose(1, 0, 2)).astype(bf)
    w3 = np.ascontiguousarray(W3.reshape(4, 128, T_ALL).transpose(1, 0, 2)).astype(bf)
    b1c = np.ascontiguousarray(b1.reshape(4, 128).T).astype(np.float32)
    b2c = np.ascontiguousarray(b2.reshape(4, 128).T).astype(np.float32)
    b3c = b3.reshape(1, T_ALL).astype(bf)
    return w1, w2, w3, b1c, b2c, b3c


def kernel(context, x, W1, b1, W2, b2, W3, b3):
    context = np.asarray(context, dtype=np.float32)
    x = np.asarray(x, dtype=np.float32)
    w1, w2, w3, b1c, b2c, b3c = _prep_weights(
        np.asarray(W1, np.float32), np.asarray(b1, np.float32),
        np.asarray(W2, np.float32), np.asarray(b2, np.float32),
        np.asarray(W3, np.float32), np.asarray(b3, np.float32))

    ctx_flat = context.reshape(B * N, C_IN)
    x_flat = x.reshape(B * N)

    in_maps = []
    for c in range(N_CORES):
        lo, hi = c * ROWS_CORE, (c + 1) * ROWS_CORE
        ctxT, xv = _prep_core(ctx_flat[lo:hi], x_flat[lo:hi])
        in_maps.append({
            "ctxT": ctxT, "xv": xv,
            "w1": w1, "w2": w2, "w3": w3,
            "b1c": b1c, "b2c": b2c, "b3c": b3c,
        })

    nc = _get_nc()
    res = run_bass_kernel_spmd(nc, in_maps, list(range(N_CORES)))

    out = np.empty((B * N,), np.float32)
    ldet = np.empty((B * N,), np.float32)
    for c in range(N_CORES):
        lo = c * ROWS_CORE
        o = res.results[c]["o"]    # [n_blk, 128, T]
        l = res.results[c]["ld"]
        out[lo:lo + ROWS_CORE] = o.transpose(0, 2, 1).reshape(-1)
        ldet[lo:lo + ROWS_CORE] = l.transpose(0, 2, 1).reshape(-1)
    return out.reshape(B, N), ldet.reshape(B, N)


if __name__ == "__main__":
    rng = np.random.default_rng(0)
    ins = {
        "context": rng.standard_normal((B, N, C_IN), dtype=np.float32),
        "x": rng.standard_normal((B, N), dtype=np.float32),
        "W1": rng.standard_normal((C_IN, D_HID), dtype=np.float32) / 16.0,
        "b1": np.zeros((D_HID,), np.float32),
        "W2": rng.standard_normal((D_HID, D_HID), dtype=np.float32) / np.sqrt(D_HID).astype(np.float32),
        "b2": np.zeros((D_HID,), np.float32),
        "W3": rng.standard_normal((D_HID, T_ALL), dtype=np.float32) / np.sqrt(D_HID).astype(np.float32),
        "b3": np.zeros((T_ALL,), np.float32),
    }
    o, l = kernel(**ins)
    print("out", o.shape, o.dtype, "ld", l.shape)


# revision 11
# speedup vs baseline: 1.0411x; 1.0411x over previous
"""Trainium2 Bass kernel for DSF marginal (MLP conditioner + deep sigmoid flow).

Contract: kernel(**inputs) takes the FULL unsharded inputs of reference.setup_inputs()
and returns (out, logdet), both [32, 4096] float32, matching reference.reference().

Sharding: data-parallel over rows (B*N = 131072) -> 8 cores x 16384 rows.
Device program (per core):
  - MLP: h1 = relu(ctx@W1+b1), h2 = relu(h1@W2+b2), params = h2@W3+b3, in bf16
    matmuls (fp32 PSUM accumulate), feature-major chaining; context arrives
    pre-transposed/cast from the host so no on-chip transposes are needed.
  - Flow: 6 sigmoid-flow layers computed with only Exp/Ln/Relu/Copy ACT
    functions (single ACT table set), elementwise work spread across
    DVE / ACT / GPSIMD.
"""

import functools

import numpy as np
import ml_dtypes

import concourse.bass as bass
import concourse.tile as tile
from concourse import mybir
from concourse.bass_utils import run_bass_kernel_spmd

F32 = mybir.dt.float32
BF16 = mybir.dt.bfloat16
AF = mybir.ActivationFunctionType
OP = mybir.AluOpType
AX = mybir.AxisListType

H = 48
N_LAYERS = 6
EPS = 1e-6
T_FLOW = 3 * H  # 144 params per layer
T_ALL = T_FLOW * N_LAYERS  # 864
C_IN = 256
D_HID = 512
B, N = 32, 4096
N_CORES = 8
ROWS_CORE = B * N // N_CORES  # 16384
SUB = 512                     # rows per matmul sub-block (PSUM N)
BLK = 2048                    # rows per flow block
T = BLK // 128                # 16 row-tiles per block
N_BLK = ROWS_CORE // BLK      # 8
N_SUB = BLK // SUB            # 4 sub-blocks per block

# This walrus build rejects instructions carrying more than one sem-wait
# ("Too many sync wait commands" in setupSyncWait). Split excess waits onto
# NOPs inserted just before the instruction on the same engine.
_MAX_WAITS = 1


def _split_excess_waits(nc):
    import bass_rust
    n_split = 0
    for f in nc.m.functions:
        for bb in f.blocks:
            out = []
            for inst in bb.instructions:
                si = getattr(inst, "sync_info", None)
                ow = list(si.on_wait) if (si is not None and si.on_wait) else []
                if len(ow) > _MAX_WAITS:
                    extra, keep = ow[:-_MAX_WAITS], ow[-_MAX_WAITS:]
                    for j in range(0, len(extra), _MAX_WAITS):
                        n_split += 1
                        nop = bass_rust.InstNoOp(
                            name=f"I-wsplit-{n_split}", engine=inst.engine,
                            ins=[], outs=[])
                        nop.sync_info = mybir.SyncInfo(
                            on_wait=list(extra[j:j + _MAX_WAITS]), on_update=[])
                        out.append(nop)
                    si.on_wait[:] = keep
                out.append(inst)
            bb.instructions = out
    return n_split


def build_nc(n_blk=N_BLK, num_devices=N_CORES, repeat=1, use_pool=False, flow=True, wait_split=True):
    nc = bass.Bass(trn_type="TRN2", num_devices=num_devices)
    tt_pool = nc.gpsimd if use_pool else nc.vector

    n_sub_total = n_blk * N_SUB
    ctxT_d = nc.dram_tensor("ctxT", [n_sub_total, 128, 2, SUB], BF16, kind="ExternalInput")
    xv_d = nc.dram_tensor("xv", [n_blk, 128, T], F32, kind="ExternalInput")
    w1_d = nc.dram_tensor("w1", [128, 2, D_HID], BF16, kind="ExternalInput")
    w2_d = nc.dram_tensor("w2", [128, 4, D_HID], BF16, kind="ExternalInput")
    w3_d = nc.dram_tensor("w3", [128, 4, T_ALL], BF16, kind="ExternalInput")
    b1_d = nc.dram_tensor("b1c", [128, 4], F32, kind="ExternalInput")
    b2_d = nc.dram_tensor("b2c", [128, 4], F32, kind="ExternalInput")
    b3_d = nc.dram_tensor("b3c", [1, T_ALL], BF16, kind="ExternalInput")
    o_d = nc.dram_tensor("o", [n_blk, 128, T], F32, kind="ExternalOutput")
    ld_d = nc.dram_tensor("ld", [n_blk, 128, T], F32, kind="ExternalOutput")

    with tile.TileContext(nc) as tc:
        with (
            tc.tile_pool(name="weights", bufs=1) as wpool,
            tc.tile_pool(name="ctx", bufs=3) as ctxp,
            tc.tile_pool(name="hbuf", bufs=2) as hpool,
            tc.tile_pool(name="params", bufs=2) as ppool,
            tc.tile_pool(name="psh", bufs=3, space="PSUM") as psh,
            tc.tile_pool(name="psp", bufs=2, space="PSUM") as psp,
            tc.tile_pool(name="flow32", bufs=1) as f32p,
            tc.tile_pool(name="flow16", bufs=2) as f16p,
            tc.tile_pool(name="rows", bufs=2) as rowp,
        ):
            w1 = wpool.tile([128, 2, D_HID], BF16, tag="w1")
            nc.sync.dma_start(out=w1[:, :, :], in_=w1_d[:, :, :])
            w2 = wpool.tile([128, 4, D_HID], BF16, tag="w2")
            nc.sync.dma_start(out=w2[:, :, :], in_=w2_d[:, :, :])
            w3 = wpool.tile([128, 4, T_ALL], BF16, tag="w3")
            nc.sync.dma_start(out=w3[:, :, :], in_=w3_d[:, :, :])
            b1s = wpool.tile([128, 4], F32, tag="b1s")
            nc.sync.dma_start(out=b1s[:, :], in_=b1_d[:, :])
            b2s = wpool.tile([128, 4], F32, tag="b2s")
            nc.sync.dma_start(out=b2s[:, :], in_=b2_d[:, :])
            b3s = wpool.tile([1, T_ALL], BF16, tag="b3s")
            nc.sync.dma_start(out=b3s[:, :], in_=b3_d[:, :])
            ones = wpool.tile([1, 128], BF16, tag="ones")
            nc.vector.memset(ones[:, :], 1.0)

            for b in [b for _ in range(repeat) for b in range(n_blk)]:
                P = ppool.tile([128, T, T_ALL], F32, tag="P")

                for s in range(N_SUB):
                    ct = ctxp.tile([128, 2, SUB], BF16, tag="ct")
                    nc.sync.dma_start(out=ct[:, :, :], in_=ctxT_d[b * N_SUB + s, :, :, :])

                    h1 = hpool.tile([128, 4, SUB], BF16, tag="h1")
                    for m in range(4):
                        ps = psh.tile([128, SUB], F32, tag="psh")
                        for k in range(2):
                            nc.tensor.matmul(
                                ps[:, :],
                                lhsT=w1[:, k, m * 128:(m + 1) * 128],
                                rhs=ct[:, k, :],
                                start=(k == 0), stop=(k == 1),
                            )
                        if m < 2:
                            nc.scalar.activation(h1[:, m, :], ps[:, :], AF.Relu,
                                                 bias=b1s[:, m:m + 1], scale=1.0)
                        else:
                            nc.vector.tensor_scalar(
                                out=h1[:, m, :], in0=ps[:, :],
                                scalar1=b1s[:, m:m + 1], scalar2=0.0,
                                op0=OP.add, op1=OP.max,
                            )

                    h2 = hpool.tile([128, 4, SUB], BF16, tag="h2")
                    for m in range(4):
                        ps = psh.tile([128, SUB], F32, tag="psh")
                        for k in range(4):
                            nc.tensor.matmul(
                                ps[:, :],
                                lhsT=w2[:, k, m * 128:(m + 1) * 128],
                                rhs=h1[:, k, :],
                                start=(k == 0), stop=(k == 3),
                            )
                        if m < 2:
                            nc.scalar.activation(h2[:, m, :], ps[:, :], AF.Relu,
                                                 bias=b2s[:, m:m + 1], scale=1.0)
                        else:
                            nc.vector.tensor_scalar(
                                out=h2[:, m, :], in0=ps[:, :],
                                scalar1=b2s[:, m:m + 1], scalar2=0.0,
                                op0=OP.add, op1=OP.max,
                            )

                    for r in range(4):
                        pp = psp.tile([128, T_ALL], F32, tag="psp")
                        for lo, hi in ((0, 512), (512, T_ALL)):
                            for k in range(4):
                                nc.tensor.matmul(
                                    pp[:, lo:hi],
                                    lhsT=h2[:, k, r * 128:(r + 1) * 128],
                                    rhs=w3[:, k, lo:hi],
                                    start=(k == 0), stop=False,
                                )
                            nc.tensor.matmul(
                                pp[:, lo:hi],
                                lhsT=ones[:, :],
                                rhs=b3s[:, lo:hi],
                                start=False, stop=True,
                            )
                        idx = s * 4 + r
                        if r < 2:
                            nc.vector.tensor_copy(out=P[:, idx, :], in_=pp[:, :])
                        else:
                            nc.scalar.copy(P[:, idx, :], pp[:, :])

                # ---- sigmoid flow over this block ----
                if not flow:
                    zz = rowp.tile([128, T], F32, tag="zz")
                    nc.vector.tensor_copy(out=zz[:, :], in_=P[:, 0, 0:T])
                    nc.sync.dma_start(out=o_d[b, :, :], in_=zz[:, :])
                    nc.sync.dma_start(out=ld_d[b, :, :], in_=zz[:, :])
                    continue
                xt = rowp.tile([128, T], F32, tag="x")
                nc.sync.dma_start(out=xt[:, :], in_=xv_d[b, :, :])
                ld = rowp.tile([128, T], F32, tag="ldet")

                xpre = None
                for i in range(N_LAYERS):
                    o0 = i * T_FLOW
                    Pa = P[:, :, o0:o0 + H]
                    Pb = P[:, :, o0 + H:o0 + 2 * H]
                    Pw = P[:, :, o0 + 2 * H:o0 + 3 * H]

                    ea = f32p.tile([128, T, H], F32, tag="ea")
                    nc.scalar.activation(ea[:, :, :], Pa, AF.Exp)
                    a = f16p.tile([128, T, H], BF16, tag="a")
                    nc.scalar.activation(a[:, :, :], ea[:, :, :], AF.Ln, bias=1.0)

                    t_ = f32p.tile([128, T, H], F32, tag="t")
                    nc.vector.tensor_tensor(
                        t_[:, :, :], a[:, :, :],
                        xt[:, :, None].broadcast_to([128, T, H]), OP.mult)
                    pre = f32p.tile([128, T, H], F32, tag="pre")
                    nc.vector.tensor_tensor(pre[:, :, :], t_[:, :, :], Pb, OP.add)

                    en = f16p.tile([128, T, H], BF16, tag="en")
                    nc.scalar.activation(en[:, :, :], pre[:, :, :], AF.Exp, scale=-1.0)
                    s_ = f32p.tile([128, T, H], F32, tag="s")
                    nc.scalar.activation(s_[:, :, :], en[:, :, :], AF.Ln, bias=1.0)
                    sig = f16p.tile([128, T, H], BF16, tag="sig")
                    nc.scalar.activation(sig[:, :, :], s_[:, :, :], AF.Exp, scale=-1.0)
                    e = f16p.tile([128, T, H], BF16, tag="e")
                    nc.scalar.activation(e[:, :, :], Pw, AF.Exp)

                    sign = f16p.tile([128, T, H], BF16, tag="sign")
                    tt_pool.tensor_tensor(sign[:, :, :], en[:, :, :], sig[:, :, :], OP.mult)
                    prod = f16p.tile([128, T, H], BF16, tag="prod")
                    nc.vector.tensor_tensor(prod[:, :, :], e[:, :, :], sig[:, :, :], OP.mult)
                    q1 = f16p.tile([128, T, H], BF16, tag="q1")
                    tt_pool.tensor_tensor(q1[:, :, :], prod[:, :, :], a[:, :, :], OP.mult)
                    ej = f16p.tile([128, T, H], BF16, tag="ej")
                    tt_pool.tensor_tensor(ej[:, :, :], q1[:, :, :], sign[:, :, :], OP.mult)

                    sum_e = rowp.tile([128, T], F32, tag="sum_e")
                    nc.vector.reduce_sum(out=sum_e[:, :], in_=e[:, :, :], axis=AX.X)
                    dot = rowp.tile([128, T], F32, tag="dot")
                    nc.vector.reduce_sum(out=dot[:, :], in_=prod[:, :, :], axis=AX.X)
                    sj = rowp.tile([128, T], F32, tag="sj")
                    nc.vector.reduce_sum(out=sj[:, :], in_=ej[:, :, :], axis=AX.X)

                    rec = rowp.tile([128, T], F32, tag="rec")
                    nc.vector.reciprocal(rec[:, :], sum_e[:, :])
                    xpre = rowp.tile([128, T], F32, tag="xpre")
                    nc.vector.tensor_mul(xpre[:, :], dot[:, :], rec[:, :])
                    u = rowp.tile([128, T], F32, tag="u")
                    nc.vector.tensor_mul(u[:, :], sj[:, :], rec[:, :])
                    lse = rowp.tile([128, T], F32, tag="lse")
                    nc.scalar.activation(lse[:, :], u[:, :], AF.Ln)
                    if i == 0:
                        nc.vector.tensor_copy(out=ld[:, :], in_=lse[:, :])
                    else:
                        nc.vector.tensor_add(ld[:, :], ld[:, :], lse[:, :])

                    if i < N_LAYERS - 1:
                        xc = rowp.tile([128, T], F32, tag="xc")
                        nc.vector.tensor_scalar(
                            out=xc[:, :], in0=xpre[:, :],
                            scalar1=1.0 - EPS, scalar2=EPS * 0.5,
                            op0=OP.mult, op1=OP.add,
                        )
                        lxc = rowp.tile([128, T], F32, tag="lxc")
                        nc.scalar.activation(lxc[:, :], xc[:, :], AF.Ln)
                        l1m = rowp.tile([128, T], F32, tag="l1m")
                        nc.scalar.activation(l1m[:, :], xc[:, :], AF.Ln, scale=-1.0, bias=1.0)
                        nc.vector.tensor_sub(xt[:, :], lxc[:, :], l1m[:, :])
                        wsum = rowp.tile([128, T], F32, tag="wsum")
                        nc.vector.tensor_add(wsum[:, :], lxc[:, :], l1m[:, :])
                        nc.vector.tensor_sub(ld[:, :], ld[:, :], wsum[:, :])

                # +5*log(1-eps) from the 5 logit layers
                nc.vector.tensor_scalar_add(ld[:, :], ld[:, :],
                                            float(5.0 * np.log1p(-EPS)))
                nc.sync.dma_start(out=o_d[b, :, :], in_=xpre[:, :])
                nc.sync.dma_start(out=ld_d[b, :, :], in_=ld[:, :])

    if wait_split:
        _split_excess_waits(nc)
    return nc


@functools.lru_cache(maxsize=2)
def _get_nc():
    return build_nc()


def _prep_core(ctx_core, x_core, n_blk=N_BLK):
    """Host-side layout: ctx_core [rows, 256] f32, x_core [rows] f32."""
    n_sub_total = n_blk * N_SUB
    ctxT = ctx_core.reshape(n_sub_total, SUB, 2, 128).transpose(0, 3, 2, 1)
    ctxT = np.ascontiguousarray(ctxT).astype(ml_dtypes.bfloat16)
    xv = np.ascontiguousarray(x_core.reshape(n_blk, T, 128).transpose(0, 2, 1))
    return ctxT, xv


def _prep_weights(W1, b1, W2, b2, W3, b3):
    bf = ml_dtypes.bfloat16
    w1 = np.ascontiguousarray(W1.reshape(2, 128, D_HID).transpose(1, 0, 2)).astype(bf)
    w2 = np.ascontiguousarray(W2.reshape(4, 128, D_HID).transpose(1, 0, 2)).astype(bf)
    w3 = np.ascontiguousarray(W3.reshape(4, 128, T_ALL).transpose(1, 0, 2)).astype(bf)
    b1c = np.ascontiguousarray(b1.reshape(4, 128).T).astype(np.float32)
    b2c = np.ascontiguousarray(b2.reshape(4, 128).T).astype(np.float32)
    b3c = b3.reshape(1, T_ALL).astype(bf)
    return w1, w2, w3, b1c, b2c, b3c


def kernel(context, x, W1, b1, W2, b2, W3, b3):
    context = np.asarray(context, dtype=np.float32)
    x = np.asarray(x, dtype=np.float32)
    w1, w2, w3, b1c, b2c, b3c = _prep_weights(
        np.asarray(W1, np.float32), np.asarray(b1, np.float32),
        np.asarray(W2, np.float32), np.asarray(b2, np.float32),
        np.asarray(W3, np.float32), np.asarray(b3, np.float32))

    ctx_flat = context.reshape(B * N, C_IN)
    x_flat = x.reshape(B * N)

    in_maps = []
    for c in range(N_CORES):
        lo, hi = c * ROWS_CORE, (c + 1) * ROWS_CORE
        ctxT, xv = _prep_core(ctx_flat[lo:hi], x_flat[lo:hi])
        in_maps.append({
            "ctxT": ctxT, "xv": xv,
            "w1": w1, "w2": w2, "w3": w3,
            "b1c": b1c, "b2c": b2c, "b3c": b3c,
        })

    nc = _get_nc()
    res = run_bass_kernel_spmd(nc, in_maps, list(range(N_CORES)))

    out = np.empty((B * N,), np.float32)
    ldet = np.empty((B * N,), np.float32)
    for c in range(N_CORES):
        lo = c * ROWS_CORE
        o = res.results[c]["o"]    # [n_blk, 128, T]
        l = res.results[c]["ld"]
        out[lo:lo + ROWS_CORE] = o.transpose(0, 2, 1).reshape(-1)
        ldet[lo:lo + ROWS_CORE] = l.transpose(0, 2, 1).reshape(-1)
    return out.reshape(B, N), ldet.reshape(B, N)


if __name__ == "__main__":
    rng = np.random.default_rng(0)
    ins = {
        "context": rng.standard_normal((B, N, C_IN), dtype=np.float32),
        "x": rng.standard_normal((B, N), dtype=np.float32),
        "W1": rng.standard_normal((C_IN, D_HID), dtype=np.float32) / 16.0,
        "b1": np.zeros((D_HID,), np.float32),
        "W2": rng.standard_normal((D_HID, D_HID), dtype=np.float32) / np.sqrt(D_HID).astype(np.float32),
        "b2": np.zeros((D_HID,), np.float32),
        "W3": rng.standard_normal((D_HID, T_ALL), dtype=np.float32) / np.sqrt(D_HID).astype(np.float32),
        "b3": np.zeros((T_ALL,), np.float32),
    }
    o, l = kernel(**ins)
    print("out", o.shape, o.dtype, "ld", l.shape)
